# revision 7
# baseline (speedup 1.0000x reference)
"""Trainium2 Bass kernel for nn_MemoryLayer (scatter_memory).

Reference computation (per token, N = B*S = 8192 tokens):
  z = x @ W_proj + b_proj                  # [N, 640]
  factor = sigmoid(2*|z|)  (== (1+tanh|z|)/2), per element
  score[n, t] = prod_l factor[n, t*10+l]   # [N, 64]
  code[n, t]  = sum_l (z[n, t*10+l] > 0) * 2^l   # bucket in [0, 1024)
  out[n] = sum_t tables[t*1024 + code[n,t]] * score[n,t] + bias

Sharding: data-parallel over tokens (1024 tokens per core, 8 cores).
Each core sees the full fp16-cast table in its own DRAM; the weighted
per-token sum is done on the tensor engine as diag(score) @ gathered_rows
accumulated in PSUM over the 64 tables.  No collectives are needed.

Device layout: tokens are processed in 8 tiles of 128; device token
d = slot*128 + partition everywhere (z, score, gather output,
accumulate, final store).

Gathering uses dma_gather (InstDMAGatherAnt): one SWDGE call fetches the
512 rows (1 MB) for one (table, half-of-tokens) pair, instead of one
indirect_dma_start per (table, 128-token tile).  The SWDGE fixed cost
(~1 us of serialized Pool-engine time per call) made the old scheme
Pool-bound at ~530 us/core; with 128 gather calls the Pool engine runs
~180 us and the kernel is HBM-bound (~128 MB of table rows per core).

dma_gather index layout: gather position j (0..num_idxs) reads its int16
index from partition j%16, column j//16 (replicated across the 8 Q7
cores, i.e. all 128 partitions), and writes the row to partition j%128,
slot j//128.  With num_idxs=512 per (t, h): j = 128*g + 16*a + p
(p<16), so idx[p, col=(g,a)] must hold code[partition 16a+p, table t,
tile 4h+g].  That 128->16 partition fold is done on the tensor engine:
8 matmuls with selection matrices S_a[p, q] = (p == 16a + q%16), which
also replicate the result across all 128 partitions for free.
"""

import numpy as np

import concourse.bacc as bacc
import concourse.bass as bass
import concourse.mybir as mybir
import concourse.tile as tile
from concourse.bass_utils import run_bass_kernel_spmd

# Problem constants (hardcoded per contest rules).
B, S = 4, 2048
HIDDEN = 1024
OUT = 1024
NUM_TABLE = 64
CODE_LEN = 10
TABLE_SIZE = 1024
TOTAL_DIM = NUM_TABLE * CODE_LEN  # 640

N_CORES = 8
N_TOKENS = B * S              # 8192
TOK = N_TOKENS // N_CORES     # 1024 tokens per core
P = 128                       # partitions
NT = TOK // P                 # 8 token tiles per core
KCH = HIDDEN // P             # 8 contraction chunks
N_HALF = 2                    # PSUM limits accumulate to 4 tiles at a time
TPH = NT // N_HALF            # tiles per half = 4
G_ROWS = TPH * P              # rows per dma_gather call = 512

dt = mybir.dt
Alu = mybir.AluOpType
Act = mybir.ActivationFunctionType
Axis = mybir.AxisListType


def emit_device_kernel(tc, out_ap, ins, dbg=None, z_fp32r=False, probe=0):
    """Emit the per-core kernel. ins is a dict name -> bass.AP.

    z_fp32r: run the projection matmul in float32r (4x faster, reduced
    mantissa).  probe: 1 = skip most accumulate matmuls + dg builds
    (DMA-floor probe); timing diagnostics only, output is wrong.
    """
    nc = tc.nc
    xT = ins["xT"]          # [1024 hidden, 1024 tok] f32 (host pre-transposed)
    W = ins["W"]            # [1024, 640] f32
    bp = ins["bproj"]       # [1, 640] f32
    tabs = ins["tabs"]      # [65536, 1024] f16
    Pm = ins["pmat"]        # [128, 640] f32  (2^l pattern, replicated rows)
    id16 = ins["id16"]      # [128, 128] f16 identity
    sfold = ins["sfold"]    # [128, 8, 128] f16 fold-selection matrices
    # out_ap: [128, 8, 1024] f32; token d = slot*128 + partition

    from contextlib import ExitStack

    with ExitStack() as ctx:
        const = ctx.enter_context(tc.tile_pool(name="const", bufs=1))

        xT_sb = const.tile([P, KCH, TOK], dt.float32)
        nc.sync.dma_start(xT_sb[:], xT[:].rearrange("(c p) h -> p c h", c=KCH))
        W_sb = const.tile([P, KCH, TOTAL_DIM], dt.float32)
        nc.sync.dma_start(W_sb[:], W[:].rearrange("(c p) h -> p c h", c=KCH))
        Pm_sb = const.tile([P, TOTAL_DIM], dt.float32)
        nc.sync.dma_start(Pm_sb[:], Pm[:])
        id16_sb = const.tile([P, P], dt.float16)
        nc.sync.dma_start(id16_sb[:], id16[:])
        sfold_sb = const.tile([P, NT, P], dt.float16)
        nc.sync.dma_start(sfold_sb[:], sfold[:])
        bp_sb = const.tile([1, TOTAL_DIM], dt.float32)
        nc.sync.dma_start(bp_sb[:], bp[:])
        ones_sb = const.tile([1, P], dt.float32)
        nc.vector.memset(ones_sb[:], 1.0)

        # persistent per-core state
        score_sb = const.tile([P, NUM_TABLE, NT], dt.float32)
        code_sb = const.tile([P, NUM_TABLE, NT], dt.float32)
        code16_sb = const.tile([P, NUM_TABLE, NT], dt.float16)
        # gather indices: [p, t, h, g, a] = code[16a + p%16, t, 4h+g]
        idx_sb = const.tile([P, NUM_TABLE, N_HALF, TPH, NT], dt.int16)

        # ---- Phase 1: projection z + score/code per token tile ----
        with tc.tile_pool(name="zp", bufs=2, space="PSUM") as zpool, \
             tc.tile_pool(name="ztmp", bufs=2) as ztmp:
            for k in range(NT):
                z = zpool.tile([P, TOTAL_DIM], dt.float32, tag="z")
                for (n0, n1) in ((0, 512), (512, TOTAL_DIM)):
                    for c in range(KCH):
                        lh = xT_sb[:, c, k * P:(k + 1) * P]
                        rh = W_sb[:, c, n0:n1]
                        if z_fp32r:
                            lh = lh.bitcast(dt.float32r)
                            rh = rh.bitcast(dt.float32r)
                        nc.tensor.matmul(
                            z[:, n0:n1],
                            lh,
                            rh,
                            start=(c == 0),
                            stop=False,
                        )
                    # + b_proj via rank-1 ones trick
                    nc.tensor.matmul(
                        z[:, n0:n1], ones_sb[0:1, :], bp_sb[0:1, n0:n1],
                        start=False, stop=True,
                    )
                ab = ztmp.tile([P, TOTAL_DIM], dt.float32, tag="ab")
                nc.scalar.activation(ab[:], z[:], Act.Abs)
                fa = ztmp.tile([P, TOTAL_DIM], dt.float32, tag="fa")
                nc.scalar.activation(fa[:], ab[:], Act.Sigmoid, scale=2.0)
                lf = ztmp.tile([P, TOTAL_DIM], dt.float32, tag="lf")
                nc.scalar.activation(lf[:], fa[:], Act.Ln)
                lsum = ztmp.tile([P, NUM_TABLE], dt.float32, tag="lsum")
                nc.vector.tensor_reduce(
                    lsum[:],
                    lf[:].rearrange("p (t l) -> p t l", l=CODE_LEN),
                    axis=Axis.X, op=Alu.add,
                )
                nc.scalar.activation(score_sb[:, :, k], lsum[:], Act.Exp)
                bc = ztmp.tile([P, TOTAL_DIM], dt.float32, tag="bc")
                nc.vector.scalar_tensor_tensor(
                    bc[:], z[:], 0.0, Pm_sb[:], op0=Alu.is_gt, op1=Alu.mult)
                nc.vector.tensor_reduce(
                    code_sb[:, :, k],
                    bc[:].rearrange("p (t l) -> p t l", l=CODE_LEN),
                    axis=Axis.X, op=Alu.add,
                )

        # ---- Phase 2: fold codes into the dma_gather index layout ----
        nc.vector.tensor_copy(code16_sb[:], code_sb[:])
        with tc.tile_pool(name="foldp", bufs=4, space="PSUM") as foldpool:
            for a in range(NT):
                fps = foldpool.tile([P, NUM_TABLE * NT], dt.float32, tag="f")
                nc.tensor.matmul(
                    fps[:], sfold_sb[:, a, :],
                    code16_sb[:].rearrange("p t k -> p (t k)"),
                    start=True, stop=True,
                )
                # fps[q, (t, k=h*4+g)] = code[16a + q%16, t, k]
                nc.vector.tensor_copy(
                    idx_sb[:, :, :, :, a],
                    fps[:].rearrange("p (t h g) -> p t h g", h=N_HALF, g=TPH),
                )

        if dbg is not None:
            nc.sync.dma_start(dbg["score"][:], score_sb[:])
            nc.sync.dma_start(dbg["code"][:], code_sb[:])
            nc.sync.dma_start(dbg["idx"][:], idx_sb[:])

        # ---- Phase 3: gather + weighted accumulate per half ----
        # One dma_gather per (half, table): 512 rows of 2 KB.  Gather
        # position j = 128g + 16a + p lands on out partition 16a+p, slot
        # g, selected by idx_sb[p, t, h, g, a].
        with tc.tile_pool(name="acc", bufs=TPH, space="PSUM") as accpool, \
             tc.tile_pool(name="gbuf", bufs=6) as gpool, \
             tc.tile_pool(name="diag", bufs=8) as dgpool, \
             tc.tile_pool(name="outs", bufs=3) as opool:
            for h in range(N_HALF):
                accs = [accpool.tile([P, OUT], dt.float32, tag="acc",
                                     name=f"acc_{h}_{a}")
                        for a in range(TPH)]
                for t in range(NUM_TABLE):
                    g_t = gpool.tile([P, TPH, OUT], dt.float16, tag="g")
                    nc.gpsimd.dma_gather(
                        out_ap=g_t[:],
                        in_ap=tabs[t * TABLE_SIZE:(t + 1) * TABLE_SIZE, :],
                        idxs_ap=idx_sb[:, t, h, :, :],
                        num_idxs=G_ROWS,
                        num_idxs_reg=G_ROWS,
                        elem_size=OUT,
                    )
                    if dbg is not None and h == 0 and t == 0:
                        nc.sync.dma_start(dbg["g0"][:], g_t[:])
                    if dbg is not None and h == 1 and t == 5:
                        nc.sync.dma_start(dbg["g1"][:], g_t[:])
                    n_acc = 1 if probe else TPH
                    for a in range(n_acc):
                        k = TPH * h + a
                        dg = dgpool.tile([P, P], dt.float16, tag="dg")
                        nc.scalar.activation(
                            dg[:], id16_sb[:], Act.Copy,
                            scale=score_sb[:, t, k:k + 1],
                        )
                        halves = ((0, 512),) if probe else ((0, 512), (512, OUT))
                        for (n0, n1) in halves:
                            nc.tensor.matmul(
                                accs[a][:, n0:n1], dg[:],
                                g_t[:, a, n0:n1],
                                start=(t == 0),
                                stop=(t == NUM_TABLE - 1),
                            )
                for a in range(TPH):
                    o_t = opool.tile([P, OUT], dt.float32, tag="o")
                    nc.vector.tensor_copy(o_t[:], accs[a][:])
                    nc.sync.dma_start(out_ap[:, TPH * h + a, :], o_t[:])


def host_inputs(hidden_states, W_proj, b_proj, tables):
    """Build the 8 per-core input maps from full problem inputs."""
    x = np.asarray(hidden_states, dtype=np.float32).reshape(N_TOKENS, HIDDEN)
    tabs16 = np.ascontiguousarray(np.asarray(tables, dtype=np.float32)
                                  .astype(np.float16))
    W = np.ascontiguousarray(np.asarray(W_proj, dtype=np.float32))
    bp = np.ascontiguousarray(np.asarray(b_proj, dtype=np.float32)[None, :])
    pow2 = (2.0 ** np.arange(CODE_LEN, dtype=np.float32))
    pmat = np.tile(np.tile(pow2, NUM_TABLE)[None, :], (P, 1)).astype(np.float32)
    pmat = np.ascontiguousarray(pmat)
    id16 = np.eye(P, dtype=np.float16)
    sfold = np.zeros((P, NT, P), dtype=np.float16)
    for a in range(NT):
        for q in range(P):
            sfold[16 * a + (q % 16), a, q] = 1.0
    in_maps = []
    for c in range(N_CORES):
        xT_c = np.ascontiguousarray(x[c * TOK:(c + 1) * TOK, :].T)
        in_maps.append({
            "xT": xT_c, "W": W, "bproj": bp, "tabs": tabs16,
            "pmat": pmat, "id16": id16, "sfold": sfold,
        })
    return in_maps


def build_nc(debug_taps=False, reps=1, z_fp32r=False, probe=0):
    nc = bacc.Bacc("TRN2", target_bir_lowering=False, debug=False)
    ins = {
        "xT": nc.dram_tensor("xT", [HIDDEN, TOK], dt.float32,
                             kind="ExternalInput").ap(),
        "W": nc.dram_tensor("W", [HIDDEN, TOTAL_DIM], dt.float32,
                            kind="ExternalInput").ap(),
        "bproj": nc.dram_tensor("bproj", [1, TOTAL_DIM], dt.float32,
                                kind="ExternalInput").ap(),
        "tabs": nc.dram_tensor("tabs", [NUM_TABLE * TABLE_SIZE, OUT],
                               dt.float16, kind="ExternalInput").ap(),
        "pmat": nc.dram_tensor("pmat", [P, TOTAL_DIM], dt.float32,
                               kind="ExternalInput").ap(),
        "id16": nc.dram_tensor("id16", [P, P], dt.float16,
                               kind="ExternalInput").ap(),
        "sfold": nc.dram_tensor("sfold", [P, NT, P], dt.float16,
                                kind="ExternalInput").ap(),
    }
    out_ap = nc.dram_tensor("out", [P, NT, OUT], dt.float32,
                            kind="ExternalOutput").ap()
    dbg = None
    if debug_taps:
        dbg = {
            "score": nc.dram_tensor("dbg_score", [P, NUM_TABLE, NT],
                                    dt.float32, kind="ExternalOutput").ap(),
            "code": nc.dram_tensor("dbg_code", [P, NUM_TABLE, NT],
                                   dt.float32, kind="ExternalOutput").ap(),
            "idx": nc.dram_tensor("dbg_idx", [P, NUM_TABLE, N_HALF, TPH, NT],
                                  dt.int16, kind="ExternalOutput").ap(),
            "g0": nc.dram_tensor("dbg_g0", [P, TPH, OUT], dt.float16,
                                 kind="ExternalOutput").ap(),
            "g1": nc.dram_tensor("dbg_g1", [P, TPH, OUT], dt.float16,
                                 kind="ExternalOutput").ap(),
        }
    with tile.TileContext(nc) as tc:
        for _ in range(reps):
            emit_device_kernel(tc, out_ap, ins, dbg=dbg, z_fp32r=z_fp32r,
                               probe=probe)
    nc.compile()
    return nc


_NC_CACHE = {}


def kernel(hidden_states, W_proj, b_proj, tables, bias, _trace=False):
    if "nc" not in _NC_CACHE:
        _NC_CACHE["nc"] = build_nc()
    nc = _NC_CACHE["nc"]
    in_maps = host_inputs(hidden_states, W_proj, b_proj, tables)
    res = run_bass_kernel_spmd(nc, in_maps, core_ids=list(range(N_CORES)),
                               trace=_trace)
    _NC_CACHE["last_results"] = res
    bias_f = np.asarray(bias, dtype=np.float32)
    parts = []
    for c in range(N_CORES):
        o = res.results[c]["out"]  # [128, 8, 1024], token d = slot*128+part
        parts.append(np.transpose(o, (1, 0, 2)).reshape(TOK, OUT))
    full = np.concatenate(parts, axis=0) + bias_f[None, :]
    return full.reshape(B, S, OUT).astype(np.float32)
